# revision 1
# baseline (speedup 1.0000x reference)
"""Trainium2 Bass kernel for nn_CorrProductBlock (equivariant product basis block).

Node-parallel across 8 NeuronCores, ~12800 nodes/core in 25 tiles of 512.

Design vs the f32 baseline:
- Host pre-transposes node features into a feature-major fp16 layout packed
  per tile ([128 ch, tile, 4 irrep-slots, 512 nodes]), so the device does no
  PE transposes and HBM traffic is halved (fp16 in / fp16 out).
- Output is produced feature-major ([ch, 4, n] per tile), written fp16, and
  un-transposed on the host.
- Per-element weight gather by one-hot matmul into *paired* PSUM banks so one
  DVE tensor-tensor consumes two gather planes per instruction (PSUM f32
  reads are stuck at 1x; pairing amortizes the fixed overhead).
- h is computed in two 2-bank PSUM halves so the gather pool can double
  buffer (pg bufs=2) within the 8-bank budget, decoupling the per-tile
  PE->DVE gather chain across tiles.
- Engine balance per tile (cost-model): PE ~3.9us (18 matmuls), DVE ~3.7us
  (3 PSUM TTs + 1 u-evac), ACT ~3.6us (h evac + 3 u-evacs), GPSIMD ~2.6us
  (sq, ss, ss2, a0a, [a1]), DMA ~3.3us.
"""

import numpy as np

import concourse.bass as bass
import concourse.bacc as bacc
import concourse.mybir as mybir
import concourse.tile as tile
from concourse.bass_utils import run_bass_kernel_spmd

MUL = 128
NUM_ELEM = 64
N_CORES = 8
N_NODES = 100000
TILE_N = 512

F32 = mybir.dt.float32
F16 = mybir.dt.float16

MULT = mybir.AluOpType.mult
ADD = mybir.AluOpType.add

# engine-assignment / structure knobs (tuned via timeline-sim sweeps)
CFG = dict(
    split_h=True,       # h in two 2-bank halves; enables pg bufs=2
    pg_bufs=2,
    sq_eng="vector",
    sq_split=False,     # sq plane 0 on gpsimd, planes 1-2 on sq_eng
    a1_eng="vector",
    a0a_eng="gpsimd",
    ucopy_engs=("scalar", "scalar"),
    u_pair=True,        # pu as [128,2,T] pairs, one ACT evac per pair
    fuse_a0=True,       # a0 = a0a + z on gpsimd; saves one Wco0 matmul
    pair_gather=True,   # K=64 gather matmuls packed 2-per-PE-pass via
                        # tile_position row halves (needs doubled one-hot)
    merge_pu=False,     # u pairs allocate from the pg pool; ph gets bufs=2
    deep=False,         # 3-generation software pipeline emission
    xin_bufs=3,
    sbuf_bufs=3,
)


def _bcast_mid(ap, k):
    """[128, T] AP -> [128, k, T] broadcast along a new middle dim."""
    return bass.AP(tensor=ap.tensor, offset=ap.offset,
                   ap=[ap.ap[0], [0, k], ap.ap[-1]])


def _tt(nc, eng, out, in0, in1, op):
    e = getattr(nc, eng)
    e.tensor_tensor(out=out, in0=in0, in1=in1, op=op)


def _build(ntiles: int, repeat: int = 1, cfg=None):
    """Build the per-core Bass program for `ntiles` tiles of TILE_N nodes.

    repeat>1 wraps the pipeline in a device-side loop (timing amplification
    only — reprocesses the same data).
    """
    c = dict(CFG)
    if cfg:
        c.update(cfg)
    nc = bacc.Bacc(num_devices=N_CORES)

    oh_rows = 128 if c["pair_gather"] else NUM_ELEM
    xf = nc.dram_tensor("xf", [128, ntiles * 4 * TILE_N], F16, kind="ExternalInput")
    ohd = nc.dram_tensor("ohd", [oh_rows, ntiles * TILE_N], F16, kind="ExternalInput")
    wpre0 = nc.dram_tensor("wpre0", [MUL, MUL], F16, kind="ExternalInput")
    wpre1 = nc.dram_tensor("wpre1", [MUL, MUL], F16, kind="ExternalInput")
    wco0 = nc.dram_tensor("wco0", [MUL, MUL], F16, kind="ExternalInput")
    wco1 = nc.dram_tensor("wco1", [MUL, MUL], F16, kind="ExternalInput")
    wsc0 = nc.dram_tensor("wsc0", [MUL, MUL], F16, kind="ExternalInput")
    wsc1 = nc.dram_tensor("wsc1", [MUL, MUL], F16, kind="ExternalInput")
    t10 = nc.dram_tensor("t10", [NUM_ELEM, MUL], F16, kind="ExternalInput")
    t11 = nc.dram_tensor("t11", [NUM_ELEM, MUL], F16, kind="ExternalInput")
    t200 = nc.dram_tensor("t200", [NUM_ELEM, MUL], F16, kind="ExternalInput")
    t211 = nc.dram_tensor("t211", [NUM_ELEM, MUL], F16, kind="ExternalInput")
    t201 = nc.dram_tensor("t201", [NUM_ELEM, MUL], F16, kind="ExternalInput")
    y = nc.dram_tensor("y", [128, ntiles * 4 * TILE_N], F16, kind="ExternalOutput")

    with tile.TileContext(nc) as tc:
        with (
            tc.tile_pool(name="singles", bufs=1) as singles,
            tc.tile_pool(name="xin", bufs=c["xin_bufs"]) as xin_pool,
            tc.tile_pool(name="ohp", bufs=c["xin_bufs"]) as oh_pool,
            tc.tile_pool(name="cc", bufs=c["sbuf_bufs"]) as cc_pool,
            tc.tile_pool(name="sqp", bufs=c["sbuf_bufs"]) as sq_pool,
            tc.tile_pool(name="ssp", bufs=c["sbuf_bufs"]) as ss_pool,
            tc.tile_pool(name="tpp", bufs=c["sbuf_bufs"]) as tp_pool,
            tc.tile_pool(name="tqp", bufs=c["sbuf_bufs"]) as tq_pool,
            tc.tile_pool(name="zzp", bufs=c["sbuf_bufs"]) as zz_pool,
            tc.tile_pool(name="a0p", bufs=c["sbuf_bufs"]) as a0_pool,
            tc.tile_pool(name="a1p", bufs=c["sbuf_bufs"]) as a1_pool,
            tc.tile_pool(name="outp", bufs=c["sbuf_bufs"]) as out_pool,
            tc.tile_pool(name="ph", bufs=2 if c["merge_pu"] else 1,
                         space="PSUM") as ph_pool,
            tc.tile_pool(name="pg", bufs=c["pg_bufs"], space="PSUM") as pg_pool,
            tc.tile_pool(name="pu", bufs=1 if c["u_pair"] else 2,
                         space="PSUM") as pu_pool,
        ):
            def load_w(dram, p, tag):
                t = singles.tile([p, MUL], F16, tag=tag)
                nc.sync.dma_start(out=t, in_=dram[:, :])
                return t

            W_pre0 = load_w(wpre0, 128, "wpre0")
            W_pre1 = load_w(wpre1, 128, "wpre1")
            W_co0 = load_w(wco0, 128, "wco0")
            W_co1 = load_w(wco1, 128, "wco1")
            W_sc0 = load_w(wsc0, 128, "wsc0")
            W_sc1 = load_w(wsc1, 128, "wsc1")
            if c["pair_gather"]:
                # stacked pairs in one [128,128] tile: rows 0-63 / 64-127
                PA = singles.tile([128, MUL], F16, tag="pa")  # [t200; t201]
                nc.sync.dma_start(out=PA[0:64, :], in_=t200[:, :])
                nc.sync.dma_start(out=PA[64:128, :], in_=t201[:, :])
                PB = singles.tile([128, MUL], F16, tag="pb")  # [t10; t11]
                nc.sync.dma_start(out=PB[0:64, :], in_=t10[:, :])
                nc.sync.dma_start(out=PB[64:128, :], in_=t11[:, :])
                T_211 = load_w(t211, 64, "t211")
            else:
                T_10 = load_w(t10, 64, "t10")
                T_11 = load_w(t11, 64, "t11")
                T_200 = load_w(t200, 64, "t200")
                T_211 = load_w(t211, 64, "t211")
                T_201 = load_w(t201, 64, "t201")

            xf_t = xf.rearrange("p (t q n) -> t p q n", t=ntiles, q=4)
            ohd_t = ohd.rearrange("e (t n) -> t e n", t=ntiles)
            y_t = y.rearrange("p (t q n) -> t p q n", t=ntiles, q=4)

            st = [dict() for _ in range(ntiles)]

            def ok(i):
                return 0 <= i < ntiles

            def stage_load(i):
                if not ok(i):
                    return
                xb = xin_pool.tile([128, 4, TILE_N], F16, tag="xb")
                nc.sync.dma_start(out=xb, in_=xf_t[i])
                st[i]["xb"] = xb

            def stage_oh(i):
                if not ok(i):
                    return
                oh = oh_pool.tile([oh_rows, TILE_N], F16, tag="oh")
                nc.sync.dma_start(out=oh, in_=ohd_t[i])
                st[i]["oh"] = oh

            # --- h = Wpre . x : two 2-bank halves (split_h) or one 4-bank ---
            def stage_pre_a(i):
                if not ok(i):
                    return
                xb = st[i]["xb"]
                if c["split_h"]:
                    h = ph_pool.tile([128, 2, TILE_N], F32, tag="h")
                    nc.tensor.matmul(h[:, 0, :], W_pre0, xb[:, 0, :],
                                     start=True, stop=True)
                    nc.tensor.matmul(h[:, 1, :], W_pre1, xb[:, 1, :],
                                     start=True, stop=True)
                    st[i]["ha"] = h
                else:
                    h = ph_pool.tile([128, 4, TILE_N], F32, tag="h")
                    nc.tensor.matmul(h[:, 0, :], W_pre0, xb[:, 0, :],
                                     start=True, stop=True)
                    for k in range(3):
                        nc.tensor.matmul(h[:, 1 + k, :], W_pre1, xb[:, 1 + k, :],
                                         start=True, stop=True)
                    st[i]["ha"] = h

            def stage_evac_a(i):
                if not ok(i):
                    return
                ct = cc_pool.tile([128, 4, TILE_N], F16, tag="cc")
                if c["split_h"]:
                    nc.scalar.copy(out=ct[:, 0:2, :], in_=st[i]["ha"])
                else:
                    nc.scalar.copy(out=ct, in_=st[i]["ha"])
                st[i]["cc"] = ct

            def stage_pre_b(i):
                if not ok(i) or not c["split_h"]:
                    return
                xb = st[i]["xb"]
                h = ph_pool.tile([128, 2, TILE_N], F32, tag="h")
                nc.tensor.matmul(h[:, 0, :], W_pre1, xb[:, 2, :], start=True, stop=True)
                nc.tensor.matmul(h[:, 1, :], W_pre1, xb[:, 3, :], start=True, stop=True)
                st[i]["hb"] = h

            def stage_evac_b(i):
                if not ok(i) or not c["split_h"]:
                    return
                nc.scalar.copy(out=st[i]["cc"][:, 2:4, :], in_=st[i]["hb"])

            def stage_sq(i):
                if not ok(i):
                    return
                ct = st[i]["cc"]
                sq = sq_pool.tile([128, 3, TILE_N], F16, tag="sq")
                if c["sq_split"]:
                    nc.gpsimd.tensor_tensor(out=sq[:, 0, :], in0=ct[:, 1, :],
                                            in1=ct[:, 1, :], op=MULT)
                    _tt(nc, c["sq_eng"], sq[:, 1:3, :], ct[:, 2:4, :],
                        ct[:, 2:4, :], MULT)
                else:
                    _tt(nc, c["sq_eng"], sq, ct[:, 1:4, :], ct[:, 1:4, :], MULT)
                st[i]["sq"] = sq

            def stage_ss(i):
                if not ok(i):
                    return
                sq = st[i]["sq"]
                ss = ss_pool.tile([128, TILE_N], F16, tag="ss")
                nc.gpsimd.tensor_add(ss, sq[:, 0, :], sq[:, 1, :])
                ss2 = ss_pool.tile([128, TILE_N], F16, tag="ss2")
                nc.gpsimd.tensor_add(ss2, ss, sq[:, 2, :])
                st[i]["ss2"] = ss2

            def _gpair(g, P, oh):
                nc.tensor.matmul(g[:, 0, :], P[0:64, :], oh[0:64, :],
                                 start=True, stop=True, tile_position=(0, 0))
                nc.tensor.matmul(g[:, 1, :], P[64:128, :], oh[64:128, :],
                                 start=True, stop=True, tile_position=(64, 0))

            def stage_gA(i):
                if not ok(i):
                    return
                oh = st[i]["oh"]
                g = pg_pool.tile([128, 2, TILE_N], F32, tag="g")
                if c["pair_gather"]:
                    _gpair(g, PA, oh)
                else:
                    nc.tensor.matmul(g[:, 0, :], T_200, oh, start=True, stop=True)
                    nc.tensor.matmul(g[:, 1, :], T_201, oh, start=True, stop=True)
                st[i]["gA"] = g

            def stage_V1(i):
                # [t1; p1] = gA * c0
                if not ok(i):
                    return
                ct = st[i]["cc"]
                tp = tp_pool.tile([128, 2, TILE_N], F16, tag="tp")
                nc.vector.tensor_tensor(out=tp, in0=st[i]["gA"],
                                        in1=_bcast_mid(ct[:, 0, :], 2), op=MULT)
                st[i]["tp"] = tp

            def stage_gB(i):
                if not ok(i):
                    return
                oh = st[i]["oh"]
                g = pg_pool.tile([128, 2, TILE_N], F32, tag="g")
                if c["pair_gather"]:
                    _gpair(g, PB, oh)
                else:
                    nc.tensor.matmul(g[:, 0, :], T_10, oh, start=True, stop=True)
                    nc.tensor.matmul(g[:, 1, :], T_11, oh, start=True, stop=True)
                st[i]["gB"] = g

            def stage_V2(i):
                # [t2; p2] = gB + [t1; p1]
                if not ok(i):
                    return
                tq = tq_pool.tile([128, 2, TILE_N], F16, tag="tq")
                nc.vector.tensor_tensor(out=tq, in0=st[i]["gB"], in1=st[i]["tp"],
                                        op=ADD)
                st[i]["tq"] = tq

            def stage_gC(i):
                if not ok(i):
                    return
                oh = st[i]["oh"]
                g = pg_pool.tile([128, 2, TILE_N], F32, tag="g")
                nc.tensor.matmul(g[:, 0, :], T_211, oh[0:64, :],
                                 start=True, stop=True)
                st[i]["gC"] = g

            def stage_V3(i):
                # z = g211 * ss2
                if not ok(i):
                    return
                zz = zz_pool.tile([128, TILE_N], F16, tag="zz")
                nc.vector.tensor_tensor(out=zz, in0=st[i]["gC"][:, 0, :],
                                        in1=st[i]["ss2"], op=MULT)
                st[i]["zz"] = zz

            def stage_a0a(i):
                # a0a = c0 * t2
                if not ok(i):
                    return
                ct = st[i]["cc"]
                a0a = a0_pool.tile([128, TILE_N], F16, tag="a0a")
                _tt(nc, c["a0a_eng"], a0a, ct[:, 0, :], st[i]["tq"][:, 0, :], MULT)
                st[i]["a0a"] = a0a

            def stage_a0(i):
                # fused a0 = a0a + z (gpsimd) -> single Wco0 matmul in final
                if not ok(i) or not c["fuse_a0"]:
                    return
                a0 = a0_pool.tile([128, TILE_N], F16, tag="a0")
                nc.gpsimd.tensor_add(a0, st[i]["a0a"], st[i]["zz"])
                st[i]["a0"] = a0

            def stage_a1(i):
                # a1 = p2 * c1
                if not ok(i):
                    return
                ct = st[i]["cc"]
                a1 = a1_pool.tile([128, 3, TILE_N], F16, tag="a1")
                _tt(nc, c["a1_eng"], a1, _bcast_mid(st[i]["tq"][:, 1, :], 3),
                    ct[:, 1:4, :], MULT)
                st[i]["a1"] = a1

            def _skip_mm(u_ap, q, xb):
                W = W_sc0 if q == 0 else W_sc1
                nc.tensor.matmul(u_ap, W, xb[:, q, :], start=True, stop=False)

            def _prod_mms(u_ap, q, s, last=True):
                # product-path accumulation for output slot q
                if q == 0:
                    if c["fuse_a0"]:
                        nc.tensor.matmul(u_ap, W_co0, s["a0"], start=False,
                                         stop=True)
                    else:
                        nc.tensor.matmul(u_ap, W_co0, s["a0a"], start=False,
                                         stop=False)
                        nc.tensor.matmul(u_ap, W_co0, s["zz"], start=False,
                                         stop=True)
                else:
                    nc.tensor.matmul(u_ap, W_co1, s["a1"][:, q - 1, :],
                                     start=False, stop=True)

            def stage_final(i):
                if not ok(i):
                    return
                s = st[i]
                xb = s["xb"]
                out_t = out_pool.tile([128, 4, TILE_N], F16, tag="out")
                if c["u_pair"] or c["merge_pu"]:
                    # pair (q2,q3) first: it only needs a1; gives the fused a0
                    # chain latency cover. Weight-major inside each pair.
                    for p, (qa, qb) in enumerate(((2, 3), (0, 1))):
                        if c["merge_pu"]:
                            u = pg_pool.tile([128, 2, TILE_N], F32, tag="g")
                        else:
                            u = pu_pool.tile([128, 2, TILE_N], F32, tag="u")
                        _skip_mm(u[:, 0, :], qa, xb)
                        if qa != 0 and qb != 0:
                            _skip_mm(u[:, 1, :], qb, xb)
                            _prod_mms(u[:, 0, :], qa, s)
                            _prod_mms(u[:, 1, :], qb, s)
                        else:
                            _prod_mms(u[:, 0, :], qa, s)
                            _skip_mm(u[:, 1, :], qb, xb)
                            _prod_mms(u[:, 1, :], qb, s)
                        eng = c["ucopy_engs"][p]
                        dst = out_t[:, qa:qb + 1, :]
                        if eng == "scalar":
                            nc.scalar.copy(out=dst, in_=u)
                        else:
                            nc.vector.tensor_copy(out=dst, in_=u)
                else:
                    for q in range(4):
                        u = pu_pool.tile([128, TILE_N], F32, tag="u")
                        _skip_mm(u, q, xb)
                        _prod_mms(u, q, s)
                        eng = c["ucopy_engs"][q]
                        if eng == "scalar":
                            nc.scalar.copy(out=out_t[:, q, :], in_=u)
                        else:
                            nc.vector.tensor_copy(out=out_t[:, q, :], in_=u)
                nc.sync.dma_start(out=y_t[i], in_=out_t)
                st[i].clear()

            def body_shallow():
                # V3(i) is emitted one iteration late so the DVE never queues
                # behind the sq->ss->ss2 GPSIMD chain of the same tile.
                for j in range(min(2, ntiles)):
                    stage_load(j)
                    stage_oh(j)
                stage_pre_a(0)
                stage_evac_a(0)
                stage_pre_b(0)
                stage_evac_b(0)
                for i in range(ntiles + 1):
                    stage_load(i + 2)
                    stage_oh(i + 2)
                    stage_gA(i)
                    stage_V1(i)
                    stage_V3(i - 1)
                    stage_a0(i - 1)
                    stage_final(i - 1)
                    stage_pre_a(i + 1)
                    stage_evac_a(i + 1)
                    stage_sq(i)
                    stage_ss(i)
                    stage_pre_b(i + 1)
                    stage_evac_b(i + 1)
                    stage_gB(i)
                    stage_V2(i)
                    stage_a1(i)
                    stage_a0a(i)
                    stage_gC(i)

            def body_deep():
                # 3-generation pipeline: pre@i+2, gathers+elementwise@i+1,
                # V3+final@i. Every producer->consumer hop has >= 1 tile of
                # slack, so no engine queues behind an unready op.
                for j in range(min(3, ntiles)):
                    stage_load(j)
                    stage_oh(j)
                for j in range(min(2, ntiles)):
                    stage_pre_a(j)
                    stage_evac_a(j)
                    stage_pre_b(j)
                    stage_evac_b(j)
                # tile 0's gather/elementwise prologue
                stage_gA(0)
                stage_V1(0)
                stage_sq(0)
                stage_ss(0)
                stage_gB(0)
                stage_V2(0)
                stage_a1(0)
                stage_a0a(0)
                stage_gC(0)
                for i in range(ntiles + 1):
                    stage_load(i + 3)
                    stage_oh(i + 3)
                    stage_V3(i)
                    stage_a0(i)
                    stage_gA(i + 1)
                    stage_final(i)
                    stage_V1(i + 1)
                    stage_pre_a(i + 2)
                    stage_evac_a(i + 2)
                    stage_sq(i + 1)
                    stage_ss(i + 1)
                    stage_pre_b(i + 2)
                    stage_evac_b(i + 2)
                    stage_gB(i + 1)
                    stage_V2(i + 1)
                    stage_a1(i + 1)
                    stage_a0a(i + 1)
                    stage_gC(i + 1)

            def body():
                if c["deep"]:
                    body_deep()
                else:
                    body_shallow()

            if repeat > 1:
                with tc.For_i(0, repeat, hint_engines=tuple(mybir.ALL_ENGINES)):
                    body()
            else:
                body()

    nc.compile()
    return nc


# ---------------------------------------------------------------- host side

def _prep_weights(inp):
    s = 1.0 / np.sqrt(MUL)
    s3 = 1.0 / np.sqrt(3.0)
    f = lambda a: np.asarray(a, dtype=np.float32)
    h = lambda a: np.ascontiguousarray(a.astype(np.float16))
    w = {}
    w["wpre0"] = h(f(inp["Wpre0"]) * s)
    w["wpre1"] = h(f(inp["Wpre1"]) * s)
    w["wco0"] = h((f(inp["Wprod0"]) @ f(inp["Wout0"])) * (s * s))
    w["wco1"] = h((f(inp["Wprod1"]) @ f(inp["Wout1"])) * (s * s))
    w["wsc0"] = h(f(inp["Wsc0"]) * s)
    w["wsc1"] = h(f(inp["Wsc1"]) * s)
    w["t10"] = h(f(inp["w1_0"]))
    w["t11"] = h(f(inp["w1_1"]))
    w["t200"] = h(f(inp["w2_00"]))
    w["t211"] = h(f(inp["w2_11"]) * s3)
    w["t201"] = h(f(inp["w2_01"]))
    return w


def _make_in_maps(node_feats, node_elems, weights, ntiles):
    """Pack full inputs into per-core feature-major fp16 DRAM images."""
    per_core = ntiles * TILE_N
    n_nodes = node_feats.shape[0]
    per_core_raw = (n_nodes + N_CORES - 1) // N_CORES

    xp = np.zeros((N_CORES, per_core, 512), dtype=np.float32)
    oh = np.zeros((N_CORES, NUM_ELEM, per_core), dtype=np.float16)
    for cix in range(N_CORES):
        lo = cix * per_core_raw
        hi = min(n_nodes, lo + per_core_raw)
        cnt = max(0, hi - lo)
        if cnt:
            xp[cix, :cnt] = node_feats[lo:hi]
            e = np.asarray(node_elems[lo:hi]).astype(np.int64)
            oh[cix, e, np.arange(cnt)] = 1.0
    if CFG["pair_gather"]:
        # doubled one-hot: rows 64-127 repeat rows 0-63 (PE row-half packing)
        oh = np.concatenate([oh, oh], axis=1)

    # x0: [C, nt, 512, 128] -> [C, 128, nt, 512]
    x0t = xp[:, :, :MUL].reshape(N_CORES, ntiles, TILE_N, MUL)
    x0t = x0t.transpose(0, 3, 1, 2)
    # x1: [C, nt, 512, 128, 3] -> [C, 128, nt, 3, 512]
    x1t = xp[:, :, MUL:].reshape(N_CORES, ntiles, TILE_N, MUL, 3)
    x1t = x1t.transpose(0, 3, 1, 4, 2)
    xq = np.empty((N_CORES, 128, ntiles, 4, TILE_N), dtype=np.float16)
    xq[:, :, :, 0, :] = x0t
    xq[:, :, :, 1:4, :] = x1t
    xq = np.ascontiguousarray(xq.reshape(N_CORES, 128, ntiles * 4 * TILE_N))
    oh = np.ascontiguousarray(oh)

    return [
        {"xf": xq[cix], "ohd": oh[cix], **weights} for cix in range(N_CORES)
    ]


def _unpack_output(res, n_nodes, ntiles):
    per_core = ntiles * TILE_N
    per_core_raw = (n_nodes + N_CORES - 1) // N_CORES
    out = np.empty((n_nodes, 512), dtype=np.float32)
    for cix in range(N_CORES):
        lo = cix * per_core_raw
        hi = min(n_nodes, lo + per_core_raw)
        cnt = max(0, hi - lo)
        if not cnt:
            continue
        yc = res.results[cix]["y"].reshape(128, ntiles, 4, TILE_N)
        u0 = yc[:, :, 0, :].reshape(128, per_core).T  # [n, 128]
        u1 = yc[:, :, 1:4, :].transpose(1, 3, 0, 2).reshape(per_core, 3 * MUL)
        out[lo:hi, :MUL] = u0[:cnt]
        out[lo:hi, MUL:] = u1[:cnt]
    return out


_cache = {}


def _get_program(ntiles, repeat=1, cfg=None):
    key = (ntiles, repeat, tuple(sorted((cfg or {}).items())))
    if key not in _cache:
        _cache[key] = _build(ntiles, repeat=repeat, cfg=cfg)
    return _cache[key]


def _run(nc, in_maps, trace=False):
    return run_bass_kernel_spmd(
        nc, in_maps, core_ids=list(range(N_CORES)), trace=trace
    )


def kernel(**inputs):
    inputs = {k: np.asarray(v) for k, v in inputs.items()}
    node_feats = inputs["node_feats"].astype(np.float32, copy=False)
    node_elems = inputs["node_elems"]
    n_nodes = node_feats.shape[0]
    per_core_raw = (n_nodes + N_CORES - 1) // N_CORES
    ntiles = (per_core_raw + TILE_N - 1) // TILE_N

    weights = _prep_weights(inputs)
    in_maps = _make_in_maps(node_feats, node_elems, weights, ntiles)
    nc = _get_program(ntiles)
    res = _run(nc, in_maps)
    return _unpack_output(res, n_nodes, ntiles)

